# revision 32
# baseline (speedup 1.0000x reference)
"""Sliding-window (tau=32) multi-head attention block with shared qkv projection,
distributed over 8 trn2 NeuronCores.

Sharding: data/sequence-parallel over the flattened (batch, token) axis —
8 shards of 1024 tokens. Each core receives its k/v slice with a 32-row
front halo (zeros at batch start), so projecting the concatenated buffer
reproduces the reference's pad-then-project semantics exactly (incl. bias).

Perf structure (v2):
- batched DMAs: one load per raw tensor (+tails), one const blob, one store.
- transposes + projections in f32r (1 cyc/row at free>=256 / 1.5 transpose).
- scores, exp, mask, out' matmuls in bf16 (1 cyc/row at any free size,
  vs f32r's 4 cyc/row below 256) — the small windowed matmuls dominate PE.
"""

import numpy as np
import ml_dtypes

import concourse.bacc as bacc
import concourse.bass as bass
import concourse.tile as tile
from concourse import mybir
from concourse.bass_utils import run_bass_kernel_spmd

B, N, E = 2, 4096, 256
H, TAU = 8, 32
HD = E // H
SCALING = HD**-0.5

NCORES = 8
T = B * N // NCORES  # 1024 q tokens per core
KT = T + 32  # kv rows incl. 32-row front halo
NB = KT // 32  # 33 kv blocks of 32
NQT = T // 128  # 8 q tiles of 128
NKTILE = (NB + 3) // 4  # 9 kpos tiles of (up to) 4 blocks

F32 = mybir.dt.float32
F32R = mybir.dt.float32r
BF16 = mybir.dt.bfloat16

# const blob column offsets (f32 units; bf16 payloads packed 2-per-lane)
_IDENTB0 = 0                              # ident [128,128] bf16 -> 64
_WTB0 = _IDENTB0 + 64                     # wT [2,128,256] bf16 -> 256
_WTAUGB0 = _WTB0 + 256                    # wTaug [2,128,264] bf16 -> 264
_MASK0 = _WTAUGB0 + 264                   # masks [3,128,512] bf16 -> 768
_B20 = _MASK0 + 768
_B2S0 = _B20 + 2
_ONESB0 = _B2S0 + 2                       # ones row [1,128] bf16 -> 64
_BAUGB0 = _ONESB0 + 64                    # baug row [1,264] bf16 -> 132
_IDENTF0 = _BAUGB0 + 132                  # ident [128,128] f32
_CBLOB = _IDENTF0 + 128


def _host_constants():
    """Band masks in S^T window coords: rows jj (kpos within block), cols ii
    (q within the 64-wide window); valid iff ii - jj in [0, 31]."""
    jj = np.arange(32)[:, None]
    ii = np.arange(64)[None, :]
    band = ((ii - jj >= 0) & (ii - jj <= 31)).astype(np.float32)  # [32, 64]
    band128 = np.tile(band, (4, 1))  # [128, 64]
    normal = np.repeat(band128[:, None, :], H, axis=1)  # [128, H, 64]
    first = normal.copy()
    # kpos tile 0, block m=0 (partitions 0:32): left window half (q block -1)
    # does not exist.
    first[0:32, :, 0:32] = 0.0
    last = np.zeros_like(normal)
    # kpos tile 8 holds only block m=32 (partitions 0:32); only its left
    # window half (q block 31) exists.
    last[0:32, :, 0:32] = np.repeat(band[:, 0:32][:, None, :], H, axis=1)
    masks = np.stack([normal.reshape(128, H * 64),
                      first.reshape(128, H * 64),
                      last.reshape(128, H * 64)])  # [3, 128, 512]
    return masks.astype(np.float32)


def _pack_bf16(a):
    """Pack a bf16 array (last axis even) into f32 lanes, little-endian."""
    a = np.ascontiguousarray(a.astype(ml_dtypes.bfloat16))
    u = a.view(np.uint16)
    u = u.reshape(*u.shape[:-1], u.shape[-1] // 2, 2)
    return np.ascontiguousarray(u).view(np.uint32)[..., 0].view(np.float32)


def _make_const_blob(W, b):
    WT = np.ascontiguousarray(W.T).astype(np.float32)  # [e_in, e_out]
    wT = WT.reshape(2, 128, 256)
    # augmented V weights: per-head 33-wide column groups, ones col slot = 0
    WTaug = np.zeros((256, H * 33), np.float32)
    for h in range(H):
        WTaug[:, 33 * h : 33 * h + 32] = WT[:, 32 * h : 32 * h + 32]
    wTaug = WTaug.reshape(2, 128, H * 33)
    b_aug = np.zeros((H * 33,), np.float32)
    for h in range(H):
        b_aug[33 * h : 33 * h + 32] = b[32 * h : 32 * h + 32]
        b_aug[33 * h + 32] = 1.0
    b2 = b.reshape(2, 128).astype(np.float32)

    blob = np.zeros((128, _CBLOB), np.float32)
    blob[:, _IDENTB0:_IDENTB0 + 64] = _pack_bf16(np.eye(128, dtype=np.float32))
    blob[:, _WTB0:_WTB0 + 256] = _pack_bf16(
        wT.transpose(1, 0, 2).reshape(128, 512))
    blob[:, _WTAUGB0:_WTAUGB0 + 264] = _pack_bf16(
        wTaug.transpose(1, 0, 2).reshape(128, 528))
    blob[:, _MASK0:_MASK0 + 768] = _pack_bf16(
        _host_constants().transpose(1, 0, 2).reshape(128, 1536))
    blob[:, _B20:_B20 + 2] = b2.T
    blob[:, _B2S0:_B2S0 + 2] = (SCALING * b2).T
    blob[0, _ONESB0:_ONESB0 + 64] = _pack_bf16(np.ones((1, 128), np.float32))
    blob[0, _BAUGB0:_BAUGB0 + 132] = _pack_bf16(b_aug.reshape(1, -1))
    blob[:, _IDENTF0:_IDENTF0 + 128] = np.eye(128, dtype=np.float32)
    return blob


def build_program(stage=4, reps=1, opts=None):
    _ = stage
    o = {"raw_bufs": 1, "xT_bufs": 1, "pool_mask": False,
         "ps_proj_bufs": 2, "ps_s_bufs": 1, "ps_s_pad": 512,
         "front_lag": 99, "dve_2psum": False, "pool_ofin": False,
         "pingpong": True, "pool_cast": False, "cast_whole": False,
         "m64": False, "xbar": False}
    if opts:
        o.update(opts)
    nc = bacc.Bacc("TRN2", target_bir_lowering=False)

    q_d = nc.dram_tensor("q", [T, E], F32, kind="ExternalInput")
    k_d = nc.dram_tensor("k", [KT, E], F32, kind="ExternalInput")
    v_d = nc.dram_tensor("v", [KT, E], F32, kind="ExternalInput")
    cb_d = nc.dram_tensor("cblob", [128, _CBLOB], F32, kind="ExternalInput")
    out_d = nc.dram_tensor("out", [T, E], F32, kind="ExternalOutput")

    with tile.TileContext(nc) as tc:
        with (
            tc.tile_pool(name="consts", bufs=1) as consts,
            tc.tile_pool(name="raw", bufs=o["raw_bufs"]) as raw_pool,
            tc.tile_pool(name="xT", bufs=o["xT_bufs"]) as xT_pool,
            tc.tile_pool(name="proj", bufs=1) as proj_pool,
            tc.tile_pool(name="aw", bufs=1) as aw_pool,
            tc.tile_pool(name="ofin", bufs=4) as ofin_pool,
            tc.tile_pool(name="oall", bufs=1) as oall_pool,
            tc.tile_pool(name="ps_proj", bufs=o["ps_proj_bufs"], space="PSUM") as ps_proj,
            tc.tile_pool(name="ps_s", bufs=o["ps_s_bufs"], space="PSUM") as ps_s,
            tc.tile_pool(name="ps_o", bufs=1, space="PSUM") as ps_o,
        ):
            # ---- constants: one DMA + bitcast views -----------------------
            blob = consts.tile([128, _CBLOB], F32)
            nc.sync.dma_start(out=blob, in_=cb_d.ap())
            ident = blob[:, _IDENTB0:_IDENTB0 + 64].bitcast(BF16)
            ident_f = blob[:, _IDENTF0:_IDENTF0 + 128]
            masks_sb = blob[:, _MASK0:_MASK0 + 768].bitcast(BF16).rearrange(
                "p (i w) -> p i w", i=3)  # [128, 3, 512] bf16
            b2_sb = blob[:, _B20:_B20 + 2]
            b2s_sb = blob[:, _B2S0:_B2S0 + 2]
            wT_bf = blob[:, _WTB0:_WTB0 + 256].bitcast(BF16).rearrange(
                "p (k e) -> p k e", k=2)
            wTaug_bf = blob[:, _WTAUGB0:_WTAUGB0 + 264].bitcast(BF16).rearrange(
                "p (k e) -> p k e", k=2)
            ones_bf = blob[0:1, _ONESB0:_ONESB0 + 64].bitcast(BF16)
            baug_bf = blob[0:1, _BAUGB0:_BAUGB0 + 132].bitcast(BF16)

            for _rep in range(reps):
              _ = _rep  # noqa
              # ---- batched raw loads --------------------------------------
              rawq = raw_pool.tile([128, NQT, E], F32, tag="rawq")
              rawk = raw_pool.tile([128, 9, E], F32, tag="rawk")
              rawv = raw_pool.tile([128, 9, E], F32, tag="rawv")
              nc.sync.dma_start(
                  out=rawq, in_=q_d.ap().rearrange("(c p) e -> p c e", p=128))
              nc.sync.dma_start(
                  out=rawk[:, 0:8, :],
                  in_=k_d.ap()[0:1024].rearrange("(c p) e -> p c e", p=128))
              nc.sync.dma_start(out=rawk[0:32, 8, :], in_=k_d.ap()[1024:KT])
              nc.sync.dma_start(
                  out=rawv[:, 0:8, :],
                  in_=v_d.ap()[0:1024].rearrange("(c p) e -> p c e", p=128))
              nc.sync.dma_start(out=rawv[0:32, 8, :], in_=v_d.ap()[1024:KT])

              # ---- PE transpose -> xT (f32r) ------------------------------
              xT_q = xT_pool.tile([128, 2, T], BF16, tag="xTq")
              xT_k = xT_pool.tile([128, 2, KT], BF16, tag="xTk")
              xT_v = xT_pool.tile([128, 2, KT], BF16, tag="xTv")

              qpT = proj_pool.tile([128, 2, T], BF16, tag="qpT")
              kpT = proj_pool.tile([128, 2, KT], BF16, tag="kpT")
              vpa = [
                  proj_pool.tile([128, H * 33], BF16, tag=f"vpa{i}",
                                 name=f"vpa{i}")
                  for i in range(9)
              ]

              # Front phase: interleave transposes with projections so PE has
              # independent work while ACT/DVE drain PSUM (drain latency would
              # otherwise stall the 2-buffer psp rotation every tile).
              drain_idx = [0]

              def drain(dst, src):
                  if drain_idx[0] % 2 == 0:
                      nc.scalar.activation(
                          dst, src, mybir.ActivationFunctionType.Copy)
                  else:
                      nc.vector.tensor_copy(dst, src)
                  drain_idx[0] += 1

              bfraw_pool_tiles = {}

              def transpose_pair(raw, xT, pair):
                  base = pair[0][0] * 128
                  tot = sum(pc for _, pc in pair)
                  if o["xbar"]:
                      # cast to bf16 on ACT/DVE, then XBAR DMA transpose on
                      # the idle SP queue: out[p, o, j] = in[j, 128o + p].
                      key = id(raw)
                      if key not in bfraw_pool_tiles:
                          bfraw_pool_tiles[key] = raw_pool.tile(
                              [128, 9, E], BF16,
                              tag=f"bfr{len(bfraw_pool_tiles)}", name="bfr")
                      rbf = bfraw_pool_tiles[key]
                      c0 = pair[0][0]
                      nch = len(pair)
                      if drain_idx[0] % 2 == 0:
                          nc.scalar.activation(
                              rbf[:, c0:c0 + nch, :], raw[:, c0:c0 + nch, :],
                              mybir.ActivationFunctionType.Copy)
                      else:
                          nc.vector.tensor_copy(
                              rbf[:, c0:c0 + nch, :], raw[:, c0:c0 + nch, :])
                      drain_idx[0] += 1
                      for c, pc in pair:
                          nc.sync.dma_start_transpose(
                              xT[:, :, 128 * c : 128 * c + pc],
                              rbf[:pc, c, :],
                          )
                      return
                  if not o["pool_cast"]:
                      # f32 transpose; the PSUM drain does the bf16 cast free
                      pt = ps_proj.tile([128, 512], F32, tag="psp",
                                        name="pt").rearrange(
                          "p (a b) -> p a b", a=2)
                      for j, (c, pc) in enumerate(pair):
                          rt = raw[:, c, :]
                          for oo in range(2):
                              nc.tensor.transpose(
                                  pt[:, oo, 128 * j : 128 * j + pc],
                                  rt[:pc, 128 * oo : 128 * oo + 128],
                                  ident_f[:pc, :pc],
                              )
                      drain(xT[:, :, base : base + tot], pt[:, :, :tot])
                      return
                  key = id(raw)
                  if key not in bfraw_pool_tiles:
                      bfraw_pool_tiles[key] = raw_pool.tile(
                          [128, 9, E], BF16, tag=f"bfr{len(bfraw_pool_tiles)}",
                          name="bfr")
                  rbf = bfraw_pool_tiles[key]
                  pt = ps_proj.tile([128, 1024], BF16, tag="psp",
                                    name="pt")[:, 0:512].rearrange(
                      "p (a b) -> p a b", a=2)
                  # cast on the (otherwise idle) GPSIMD: one op per tensor
                  # (coarse, but launches amortized; pipelines across reps)
                  c0 = pair[0][0]
                  nch = len(pair)
                  if o["cast_whole"]:
                      if c0 == 0:
                          nch_all = raw.shape[1]
                          nc.gpsimd.tensor_copy(
                              rbf[:, 0:nch_all, :], raw[:, 0:nch_all, :])
                  else:
                      nc.gpsimd.tensor_copy(
                          rbf[:, c0:c0 + nch, :], raw[:, c0:c0 + nch, :])
                  for j, (c, pc) in enumerate(pair):
                      rt = rbf[:, c, :]
                      for oo in range(2):
                          nc.tensor.transpose(
                              pt[:, oo, 128 * j : 128 * j + pc],
                              rt[:pc, 128 * oo : 128 * oo + 128],
                              ident[:pc, :pc],
                          )
                  drain(xT[:, :, base : base + tot], pt[:, :, :tot])

              def proj_slice(xT, outT, j, w, bias_sb, scale):
                  for o in range(2):
                      ps = ps_proj.tile([128, 512], F32, tag="psp",
                                        name="ps")
                      for ki in range(2):
                          nc.tensor.matmul(
                              ps[:, :w],
                              wT_bf[:, ki, 128 * o : 128 * o + 128],
                              xT[:, ki, j : j + w],
                              start=(ki == 0),
                              stop=(ki == 1),
                          )
                      if drain_idx[0] % 2 == 0:
                          nc.scalar.activation(
                              outT[:, o, j : j + w],
                              ps[:, :w],
                              mybir.ActivationFunctionType.Identity,
                              bias=bias_sb[:, o : o + 1],
                              scale=scale,
                          )
                      else:
                          nc.vector.tensor_scalar(
                              outT[:, o, j : j + w],
                              ps[:, :w],
                              scale,
                              bias_sb[:, o : o + 1],
                              mybir.AluOpType.mult,
                              mybir.AluOpType.add,
                          )
                      drain_idx[0] += 1

              def vaug_chunk(c0, pc, idx):
                  ps = ps_proj.tile([128, 512], F32, tag="psp",
                                    name="ps")
                  for ki in range(2):
                      nc.tensor.matmul(
                          ps[:pc, 0 : H * 33],
                          xT_v[:, ki, c0 : c0 + pc],
                          wTaug_bf[:, ki, :],
                          start=(ki == 0),
                          stop=False,
                      )
                  nc.tensor.matmul(
                      ps[:pc, 0 : H * 33],
                      ones_bf[:, :pc],
                      baug_bf,
                      start=False,
                      stop=True,
                  )
                  drain(vpa[idx][:pc, :], ps[:pc, 0 : H * 33])

              q_pairs = [[(c, 128), (c + 1, 128)] for c in range(0, 8, 2)]
              kv_pairs = q_pairs + [[(8, 32)]]
              # work items: (kind, payload); emitted so a transpose pair is
              # always in flight between dependent projection slices.
              work = []
              for p in q_pairs:
                  work.append(("t", (rawq, xT_q, p)))
              for p in kv_pairs:
                  work.append(("t", (rawk, xT_k, p)))
              for p in kv_pairs:
                  work.append(("t", (rawv, xT_v, p)))
              for j in range(0, T, 256):
                  work.append(("pq", j))
              for j in range(0, 1024, 256):
                  work.append(("pk", j))
              work.append(("pk_tail", 1024))
              kv_chunks = [(c * 128, 128) for c in range(8)] + [(1024, 32)]
              for idx, (c0, pc) in enumerate(kv_chunks):
                  work.append(("v", (c0, pc, idx)))

              # schedule: run transposes in order, inserting each projection
              # item as soon as its inputs' transposes have been emitted.
              t_items = [w for w in work if w[0] == "t"]
              # number of t-items that must precede: q slice j needs q pairs
              # up to (j+256)/256; k slice needs 4 q pairs + ...; v chunk all.
              def prereq(item):
                  kind, pl = item
                  if kind == "pq":
                      return (pl + 256) // 256
                  if kind == "pk":
                      return 4 + (pl + 256) // 256
                  if kind == "pk_tail":
                      return 9
                  if kind == "v":
                      c0, pc, idx = pl
                      return 9 + (c0 + pc + 127) // 256 + 1
                  return 0
              p_items = sorted([w for w in work if w[0] != "t"],
                               key=prereq)
              emitted_t = 0
              pi = 0
              for t_item in t_items:
                  transpose_pair(*t_item[1])
                  emitted_t += 1
                  while pi < len(p_items) and prereq(p_items[pi]) + o[
                          "front_lag"] <= emitted_t:
                      kind, pl = p_items[pi]
                      if kind == "pq":
                          proj_slice(xT_q, qpT, pl, 256, b2_sb, 1.0)
                      elif kind == "pk":
                          proj_slice(xT_k, kpT, pl, 256, b2s_sb, SCALING)
                      elif kind == "pk_tail":
                          proj_slice(xT_k, kpT, 1024, 32, b2s_sb, SCALING)
                      else:
                          vaug_chunk(*pl)
                      pi += 1
              while pi < len(p_items):
                  kind, pl = p_items[pi]
                  if kind == "pq":
                      proj_slice(xT_q, qpT, pl, 256, b2_sb, 1.0)
                  elif kind == "pk":
                      proj_slice(xT_k, kpT, pl, 256, b2s_sb, SCALING)
                  elif kind == "pk_tail":
                      proj_slice(xT_k, kpT, 1024, 32, b2s_sb, SCALING)
                  else:
                      vaug_chunk(*pl)
                  pi += 1

              # ---- scores (S^T windowed, bf16) + exp + mask ---------------
              # PSUM layout: [128 (sig,jj), 4 (hr -> bank), 128 (ht,64win)].
              aw = [
                  aw_pool.tile([128, 4, 128], BF16, tag=f"aw{c}",
                               name=f"aw{c}")
                  for c in range(NKTILE)
              ]
              ofin_all = oall_pool.tile([128, NQT, H, 32], F32, tag="oall")

              # ---- out' matmuls + normalize -------------------------------
              def out_tile(t, psf):
                  _ = psf
                  po = ps_o.tile([128, 2, H, 64], F32, tag="pso", name="po")
                  def po_ap(r0, rn, mi, h, wn):
                      return po[r0:r0 + rn, mi, h, 0:wn]
                  if o["m64"]:
                      # kv block m=4t+j covers q rows 32(j-1):32(j+1) of this
                      # tile. Odd j are 64-aligned -> one M=64 matmul; even j
                      # split into M=32 halves (tile col positions must be
                      # 0/64 for 64-wide tiles). j parity -> bank; writes
                      # within a bank are row-disjoint (no concurrent RMW).
                      for h in range(H):
                          hr, ht = h % 4, h // 4
                          # (j, out_row0, rows, lhs_half_col, width)
                          pieces = [
                              (0, 0, 32, 32, 32),
                              (1, 0, 64, 0, 64),
                              (2, 32, 32, 0, 32),
                              (2, 64, 32, 32, 32),
                              (3, 64, 64, 0, 64),
                              (4, 96, 32, 0, 32),
                          ]
                          for j, r0, rn, half, wm in pieces:
                              m = 4 * t + j
                              c, sig = m // 4, m % 4
                              lhsT = aw[c][
                                  32 * sig : 32 * sig + 32, hr,
                                  64 * ht + half : 64 * ht + half + wm,
                              ]
                              rhs = vpa[c][
                                  32 * sig : 32 * sig + 32,
                                  33 * h : 33 * h + 33
                              ]
                              nc.tensor.matmul(
                                  po_ap(r0, rn, j % 2, h, 33),
                                  lhsT,
                                  rhs,
                                  start=True,
                                  stop=True,
                                  tile_position=(32 * sig, r0 if rn == 64
                                                 else r0),
                              )
                  else:
                    for gi in range(4):
                      g = 4 * t + gi
                      for h in range(H):
                          hr, ht = h % 4, h // 4
                          for mi, m in enumerate((g, g + 1)):
                              c, sig = m // 4, m % 4
                              half = 32 if m == g else 0
                              lhsT = aw[c][
                                  32 * sig : 32 * sig + 32, hr,
                                  64 * ht + half : 64 * ht + half + 32,
                              ]
                              rhs = vpa[c][
                                  32 * sig : 32 * sig + 32, 33 * h : 33 * h + 33
                              ]
                              nc.tensor.matmul(
                                  po_ap(32 * gi, 32, mi, h, 33),
                                  lhsT,
                                  rhs,
                                  start=True,
                                  stop=True,
                                  tile_position=(32 * sig, 32 * gi),
                              )
                  def po_all(mi):
                      return po[:, mi, :, 0:33]
                  osum = ofin_pool.tile([128, H, 33], F32, tag="osum")
                  if o["dve_2psum"]:
                      nc.vector.scalar_tensor_tensor(
                          out=osum,
                          in0=po_all(0),
                          scalar=1.0,
                          in1=po_all(1),
                          op0=mybir.AluOpType.mult,
                          op1=mybir.AluOpType.add,
                      )
                  else:
                      pb_sb = ofin_pool.tile([128, H, 33], F32, tag="pb_sb")
                      nc.scalar.activation(
                          pb_sb, po_all(1), mybir.ActivationFunctionType.Copy
                      )
                      nc.vector.scalar_tensor_tensor(
                          out=osum,
                          in0=po_all(0),
                          scalar=1.0,
                          in1=pb_sb,
                          op0=mybir.AluOpType.mult,
                          op1=mybir.AluOpType.add,
                      )
                  rec = ofin_pool.tile([128, H], F32, tag="rec")
                  nc.vector.reciprocal(rec, osum[:, :, 32])
                  rec_b = bass.AP(
                      tensor=rec.tensor,
                      offset=rec.offset,
                      ap=[rec.ap[0], [rec.ap[1][0], H], [0, 32]],
                  )
                  (nc.gpsimd if o["pool_ofin"] else nc.vector).tensor_mul(
                      ofin_all[:, t], osum[:, :, 0:32], rec_b)

              psf_prev = [None]
              if o["pingpong"]:
                  pss_persist = ps_s.tile([128, 4, 512], F32, tag="pss",
                                          name="pss_persist")
              for c in range(NKTILE):
                  nsig = 4 if c < NKTILE - 1 else NB - 4 * c
                  if o["pingpong"]:
                      off = 256 * (c % 2)
                      ps = pss_persist[:, :, off:off + 128]
                      psf = ps
                  else:
                      psf = ps_s.tile([128, 4, 128], F32, tag="pss",
                                      name="psf",
                                      padded_shape=[128, 4, o["ps_s_pad"]])
                      ps = psf
                  if c == NKTILE - 1:
                      nc.vector.memset(ps[:, :, 0:128], 0.0)
                  for sig in range(nsig):
                      m = 4 * c + sig
                      for h in range(H):
                          hr, ht = h % 4, h // 4
                          lhsT = kpT[32 * hr : 32 * hr + 32, ht,
                                     32 * m : 32 * m + 32]
                          if m == 0:
                              rhs = qpT[32 * hr : 32 * hr + 32, ht, 0:32]
                              outap = ps[32 * sig : 32 * sig + 32, hr,
                                         64 * ht + 32 : 64 * ht + 64]
                          elif m == NB - 1:
                              rhs = qpT[
                                  32 * hr : 32 * hr + 32, ht,
                                  32 * (m - 1) : 32 * m
                              ]
                              outap = ps[32 * sig : 32 * sig + 32, hr,
                                         64 * ht : 64 * ht + 32]
                          else:
                              rhs = qpT[
                                  32 * hr : 32 * hr + 32, ht,
                                  32 * (m - 1) : 32 * (m + 1),
                              ]
                              outap = ps[32 * sig : 32 * sig + 32, hr,
                                         64 * ht : 64 * ht + 64]
                          nc.tensor.matmul(
                              outap,
                              lhsT,
                              rhs,
                              start=True,
                              stop=True,
                              tile_position=(32 * hr, 32 * sig),
                          )
                  # zero never-written PSUM regions so exp sees finite values
                  if c == 0:
                      nc.vector.memset(ps[0:32, :, 0:32], 0.0)
                      nc.vector.memset(ps[0:32, :, 64:96], 0.0)
                  ex = aw_pool.tile([128, 4, 128], BF16, tag="ex", bufs=3)
                  nc.scalar.activation(ex, ps[:, :, 0:128],
                                       mybir.ActivationFunctionType.Exp)
                  mi = 0 if 0 < c < NKTILE - 1 else (1 if c == 0 else 2)
                  mask_eng = nc.gpsimd if (o["pool_mask"] and c % 2 == 0) else nc.vector
                  mask_eng.tensor_mul(aw[c], ex, masks_sb[:, mi, :].rearrange(
                      "p (r w) -> p r w", r=4))
                  if c >= 1:
                      out_tile(c - 1, psf)
                  psf_prev[0] = psf

              # ---- single batched store -----------------------------------
              nc.sync.dma_start(
                  out=out_d.ap().rearrange("(t p) e -> p t e", p=128),
                  in_=ofin_all.rearrange("p t h w -> p t (h w)"),
              )

    nc.compile()
    return nc


_NC_CACHE = None


def _get_nc():
    global _NC_CACHE
    if _NC_CACHE is None:
        _NC_CACHE = build_program()
    return _NC_CACHE


def make_in_maps(query, key, value, W, b):
    query = np.asarray(query, np.float32)
    key = np.asarray(key, np.float32)
    value = np.asarray(value, np.float32)
    W = np.asarray(W, np.float32)
    b = np.asarray(b, np.float32)

    cblob = _make_const_blob(W, b)

    qf = query.reshape(B * N, E)
    kf = key.reshape(B * N, E)
    vf = value.reshape(B * N, E)
    shards_per_b = NCORES // B
    in_maps = []
    for c in range(NCORES):
        s0 = c * T
        halo0 = s0 - 32
        if c % shards_per_b == 0:
            halo_k = np.zeros((32, E), np.float32)
            halo_v = np.zeros((32, E), np.float32)
        else:
            halo_k = kf[halo0:s0]
            halo_v = vf[halo0:s0]
        in_maps.append(
            {
                "q": np.ascontiguousarray(qf[s0 : s0 + T]),
                "k": np.ascontiguousarray(np.concatenate([halo_k, kf[s0 : s0 + T]])),
                "v": np.ascontiguousarray(np.concatenate([halo_v, vf[s0 : s0 + T]])),
                "cblob": cblob,
            }
        )
    return in_maps


def kernel(query, key, value, W, b):
    nc = _get_nc()
    in_maps = make_in_maps(query, key, value, W, b)
    res = run_bass_kernel_spmd(nc, in_maps, list(range(NCORES)))
    out = np.concatenate([res.results[c]["out"] for c in range(NCORES)], axis=0)
    return out.reshape(B, N, E).astype(np.float32)


# revision 33
# speedup vs baseline: 1.3542x; 1.3542x over previous
"""Sliding-window (tau=32) multi-head attention block with shared qkv projection,
distributed over 8 trn2 NeuronCores.

Sharding: data/sequence-parallel over the flattened (batch, token) axis —
8 shards of 1024 tokens. Each core receives its k/v slice with a 32-row
front halo (zeros at batch start), so projecting the concatenated buffer
reproduces the reference's pad-then-project semantics exactly (incl. bias).

Perf structure (v2):
- batched DMAs: one load per raw tensor (+tails), one const blob, one store.
- transposes + projections in f32r (1 cyc/row at free>=256 / 1.5 transpose).
- scores, exp, mask, out' matmuls in bf16 (1 cyc/row at any free size,
  vs f32r's 4 cyc/row below 256) — the small windowed matmuls dominate PE.
"""

import numpy as np
import ml_dtypes

import concourse.bacc as bacc
import concourse.bass as bass
import concourse.tile as tile
from concourse import mybir
from concourse.bass_utils import run_bass_kernel_spmd

B, N, E = 2, 4096, 256
H, TAU = 8, 32
HD = E // H
SCALING = HD**-0.5

NCORES = 8
T = B * N // NCORES  # 1024 q tokens per core
KT = T + 32  # kv rows incl. 32-row front halo
NB = KT // 32  # 33 kv blocks of 32
NQT = T // 128  # 8 q tiles of 128
NKTILE = (NB + 3) // 4  # 9 kpos tiles of (up to) 4 blocks

F32 = mybir.dt.float32
F32R = mybir.dt.float32r
BF16 = mybir.dt.bfloat16

# const blob column offsets (f32 units; bf16 payloads packed 2-per-lane)
_IDENTB0 = 0                              # ident [128,128] bf16 -> 64
_WTB0 = _IDENTB0 + 64                     # wT [2,128,256] bf16 -> 256
_WTAUGB0 = _WTB0 + 256                    # wTaug [2,128,264] bf16 -> 264
_MASK0 = _WTAUGB0 + 264                   # masks [3,128,512] bf16 -> 768
_B20 = _MASK0 + 768
_B2S0 = _B20 + 2
_ONESB0 = _B2S0 + 2                       # ones row [1,128] bf16 -> 64
_BAUGB0 = _ONESB0 + 64                    # baug row [1,264] bf16 -> 132
_IDENTF0 = _BAUGB0 + 132                  # ident [128,128] f32
_CBLOB = _IDENTF0 + 128


def _host_constants():
    """Band masks in S^T window coords: rows jj (kpos within block), cols ii
    (q within the 64-wide window); valid iff ii - jj in [0, 31]."""
    jj = np.arange(32)[:, None]
    ii = np.arange(64)[None, :]
    band = ((ii - jj >= 0) & (ii - jj <= 31)).astype(np.float32)  # [32, 64]
    band128 = np.tile(band, (4, 1))  # [128, 64]
    normal = np.repeat(band128[:, None, :], H, axis=1)  # [128, H, 64]
    first = normal.copy()
    # kpos tile 0, block m=0 (partitions 0:32): left window half (q block -1)
    # does not exist.
    first[0:32, :, 0:32] = 0.0
    last = np.zeros_like(normal)
    # kpos tile 8 holds only block m=32 (partitions 0:32); only its left
    # window half (q block 31) exists.
    last[0:32, :, 0:32] = np.repeat(band[:, 0:32][:, None, :], H, axis=1)
    masks = np.stack([normal.reshape(128, H * 64),
                      first.reshape(128, H * 64),
                      last.reshape(128, H * 64)])  # [3, 128, 512]
    return masks.astype(np.float32)


def _pack_bf16(a):
    """Pack a bf16 array (last axis even) into f32 lanes, little-endian."""
    a = np.ascontiguousarray(a.astype(ml_dtypes.bfloat16))
    u = a.view(np.uint16)
    u = u.reshape(*u.shape[:-1], u.shape[-1] // 2, 2)
    return np.ascontiguousarray(u).view(np.uint32)[..., 0].view(np.float32)


def _make_const_blob(W, b):
    WT = np.ascontiguousarray(W.T).astype(np.float32)  # [e_in, e_out]
    wT = WT.reshape(2, 128, 256)
    # augmented V weights: per-head 33-wide column groups, ones col slot = 0
    WTaug = np.zeros((256, H * 33), np.float32)
    for h in range(H):
        WTaug[:, 33 * h : 33 * h + 32] = WT[:, 32 * h : 32 * h + 32]
    wTaug = WTaug.reshape(2, 128, H * 33)
    b_aug = np.zeros((H * 33,), np.float32)
    for h in range(H):
        b_aug[33 * h : 33 * h + 32] = b[32 * h : 32 * h + 32]
        b_aug[33 * h + 32] = 1.0
    b2 = b.reshape(2, 128).astype(np.float32)

    blob = np.zeros((128, _CBLOB), np.float32)
    blob[:, _IDENTB0:_IDENTB0 + 64] = _pack_bf16(np.eye(128, dtype=np.float32))
    blob[:, _WTB0:_WTB0 + 256] = _pack_bf16(
        wT.transpose(1, 0, 2).reshape(128, 512))
    blob[:, _WTAUGB0:_WTAUGB0 + 264] = _pack_bf16(
        wTaug.transpose(1, 0, 2).reshape(128, 528))
    blob[:, _MASK0:_MASK0 + 768] = _pack_bf16(
        _host_constants().transpose(1, 0, 2).reshape(128, 1536))
    blob[:, _B20:_B20 + 2] = b2.T
    blob[:, _B2S0:_B2S0 + 2] = (SCALING * b2).T
    blob[0, _ONESB0:_ONESB0 + 64] = _pack_bf16(np.ones((1, 128), np.float32))
    blob[0, _BAUGB0:_BAUGB0 + 132] = _pack_bf16(b_aug.reshape(1, -1))
    blob[:, _IDENTF0:_IDENTF0 + 128] = np.eye(128, dtype=np.float32)
    return blob


def build_program(stage=4, reps=1, opts=None):
    _ = stage
    o = {"raw_bufs": 1, "xT_bufs": 1, "pool_mask": False,
         "ps_proj_bufs": 2, "ps_s_bufs": 1, "ps_s_pad": 512,
         "front_lag": 99, "dve_2psum": False, "pool_ofin": False,
         "pingpong": True, "pool_cast": False, "cast_whole": False,
         "m64": False, "xbar": False, "split_exp": False}
    if opts:
        o.update(opts)
    nc = bacc.Bacc("TRN2", target_bir_lowering=False)

    q_d = nc.dram_tensor("q", [T, E], F32, kind="ExternalInput")
    k_d = nc.dram_tensor("k", [KT, E], F32, kind="ExternalInput")
    v_d = nc.dram_tensor("v", [KT, E], F32, kind="ExternalInput")
    cb_d = nc.dram_tensor("cblob", [128, _CBLOB], F32, kind="ExternalInput")
    out_d = nc.dram_tensor("out", [T, E], F32, kind="ExternalOutput")

    with tile.TileContext(nc) as tc:
        with (
            tc.tile_pool(name="consts", bufs=1) as consts,
            tc.tile_pool(name="raw", bufs=o["raw_bufs"]) as raw_pool,
            tc.tile_pool(name="xT", bufs=o["xT_bufs"]) as xT_pool,
            tc.tile_pool(name="proj", bufs=1) as proj_pool,
            tc.tile_pool(name="aw", bufs=1) as aw_pool,
            tc.tile_pool(name="ofin", bufs=4) as ofin_pool,
            tc.tile_pool(name="oall", bufs=1) as oall_pool,
            tc.tile_pool(name="ps_proj", bufs=o["ps_proj_bufs"], space="PSUM") as ps_proj,
            tc.tile_pool(name="ps_s", bufs=o["ps_s_bufs"], space="PSUM") as ps_s,
            tc.tile_pool(name="ps_o", bufs=1, space="PSUM") as ps_o,
        ):
            # ---- constants: one DMA + bitcast views -----------------------
            blob = consts.tile([128, _CBLOB], F32)
            nc.sync.dma_start(out=blob, in_=cb_d.ap())
            ident = blob[:, _IDENTB0:_IDENTB0 + 64].bitcast(BF16)
            ident_f = blob[:, _IDENTF0:_IDENTF0 + 128]
            masks_sb = blob[:, _MASK0:_MASK0 + 768].bitcast(BF16).rearrange(
                "p (i w) -> p i w", i=3)  # [128, 3, 512] bf16
            b2_sb = blob[:, _B20:_B20 + 2]
            b2s_sb = blob[:, _B2S0:_B2S0 + 2]
            wT_bf = blob[:, _WTB0:_WTB0 + 256].bitcast(BF16).rearrange(
                "p (k e) -> p k e", k=2)
            wTaug_bf = blob[:, _WTAUGB0:_WTAUGB0 + 264].bitcast(BF16).rearrange(
                "p (k e) -> p k e", k=2)
            ones_bf = blob[0:1, _ONESB0:_ONESB0 + 64].bitcast(BF16)
            baug_bf = blob[0:1, _BAUGB0:_BAUGB0 + 132].bitcast(BF16)

            for _rep in range(reps):
              _ = _rep  # noqa
              # ---- batched raw loads --------------------------------------
              rawq = raw_pool.tile([128, NQT, E], F32, tag="rawq")
              rawk = raw_pool.tile([128, 9, E], F32, tag="rawk")
              rawv = raw_pool.tile([128, 9, E], F32, tag="rawv")
              nc.sync.dma_start(
                  out=rawq, in_=q_d.ap().rearrange("(c p) e -> p c e", p=128))
              nc.sync.dma_start(
                  out=rawk[:, 0:8, :],
                  in_=k_d.ap()[0:1024].rearrange("(c p) e -> p c e", p=128))
              nc.sync.dma_start(out=rawk[0:32, 8, :], in_=k_d.ap()[1024:KT])
              nc.sync.dma_start(
                  out=rawv[:, 0:8, :],
                  in_=v_d.ap()[0:1024].rearrange("(c p) e -> p c e", p=128))
              nc.sync.dma_start(out=rawv[0:32, 8, :], in_=v_d.ap()[1024:KT])

              # ---- PE transpose -> xT (f32r) ------------------------------
              xT_q = xT_pool.tile([128, 2, T], BF16, tag="xTq")
              xT_k = xT_pool.tile([128, 2, KT], BF16, tag="xTk")
              xT_v = xT_pool.tile([128, 2, KT], BF16, tag="xTv")

              qpT = proj_pool.tile([128, 2, T], BF16, tag="qpT")
              kpT = proj_pool.tile([128, 2, KT], BF16, tag="kpT")
              vpa = [
                  proj_pool.tile([128, H * 33], BF16, tag=f"vpa{i}",
                                 name=f"vpa{i}")
                  for i in range(9)
              ]

              # Front phase: interleave transposes with projections so PE has
              # independent work while ACT/DVE drain PSUM (drain latency would
              # otherwise stall the 2-buffer psp rotation every tile).
              drain_idx = [0]

              def drain(dst, src):
                  if drain_idx[0] % 2 == 0:
                      nc.scalar.activation(
                          dst, src, mybir.ActivationFunctionType.Copy)
                  else:
                      nc.vector.tensor_copy(dst, src)
                  drain_idx[0] += 1

              bfraw_pool_tiles = {}

              def transpose_pair(raw, xT, pair):
                  base = pair[0][0] * 128
                  tot = sum(pc for _, pc in pair)
                  if o["xbar"]:
                      # cast to bf16 on ACT/DVE, then XBAR DMA transpose on
                      # the idle SP queue: out[p, o, j] = in[j, 128o + p].
                      key = id(raw)
                      if key not in bfraw_pool_tiles:
                          bfraw_pool_tiles[key] = raw_pool.tile(
                              [128, 9, E], BF16,
                              tag=f"bfr{len(bfraw_pool_tiles)}", name="bfr")
                      rbf = bfraw_pool_tiles[key]
                      c0 = pair[0][0]
                      nch = len(pair)
                      if drain_idx[0] % 2 == 0:
                          nc.scalar.activation(
                              rbf[:, c0:c0 + nch, :], raw[:, c0:c0 + nch, :],
                              mybir.ActivationFunctionType.Copy)
                      else:
                          nc.vector.tensor_copy(
                              rbf[:, c0:c0 + nch, :], raw[:, c0:c0 + nch, :])
                      drain_idx[0] += 1
                      for c, pc in pair:
                          nc.sync.dma_start_transpose(
                              xT[:, :, 128 * c : 128 * c + pc],
                              rbf[:pc, c, :],
                          )
                      return
                  if not o["pool_cast"]:
                      # f32 transpose; the PSUM drain does the bf16 cast free
                      pt = ps_proj.tile([128, 512], F32, tag="psp",
                                        name="pt").rearrange(
                          "p (a b) -> p a b", a=2)
                      for j, (c, pc) in enumerate(pair):
                          rt = raw[:, c, :]
                          for oo in range(2):
                              nc.tensor.transpose(
                                  pt[:, oo, 128 * j : 128 * j + pc],
                                  rt[:pc, 128 * oo : 128 * oo + 128],
                                  ident_f[:pc, :pc],
                              )
                      drain(xT[:, :, base : base + tot], pt[:, :, :tot])
                      return
                  key = id(raw)
                  if key not in bfraw_pool_tiles:
                      bfraw_pool_tiles[key] = raw_pool.tile(
                          [128, 9, E], BF16, tag=f"bfr{len(bfraw_pool_tiles)}",
                          name="bfr")
                  rbf = bfraw_pool_tiles[key]
                  pt = ps_proj.tile([128, 1024], BF16, tag="psp",
                                    name="pt")[:, 0:512].rearrange(
                      "p (a b) -> p a b", a=2)
                  # cast on the (otherwise idle) GPSIMD: one op per tensor
                  # (coarse, but launches amortized; pipelines across reps)
                  c0 = pair[0][0]
                  nch = len(pair)
                  if o["cast_whole"]:
                      if c0 == 0:
                          nch_all = raw.shape[1]
                          nc.gpsimd.tensor_copy(
                              rbf[:, 0:nch_all, :], raw[:, 0:nch_all, :])
                  else:
                      nc.gpsimd.tensor_copy(
                          rbf[:, c0:c0 + nch, :], raw[:, c0:c0 + nch, :])
                  for j, (c, pc) in enumerate(pair):
                      rt = rbf[:, c, :]
                      for oo in range(2):
                          nc.tensor.transpose(
                              pt[:, oo, 128 * j : 128 * j + pc],
                              rt[:pc, 128 * oo : 128 * oo + 128],
                              ident[:pc, :pc],
                          )
                  drain(xT[:, :, base : base + tot], pt[:, :, :tot])

              def proj_slice(xT, outT, j, w, bias_sb, scale):
                  for o in range(2):
                      ps = ps_proj.tile([128, 512], F32, tag="psp",
                                        name="ps")
                      for ki in range(2):
                          nc.tensor.matmul(
                              ps[:, :w],
                              wT_bf[:, ki, 128 * o : 128 * o + 128],
                              xT[:, ki, j : j + w],
                              start=(ki == 0),
                              stop=(ki == 1),
                          )
                      if drain_idx[0] % 2 == 0:
                          nc.scalar.activation(
                              outT[:, o, j : j + w],
                              ps[:, :w],
                              mybir.ActivationFunctionType.Identity,
                              bias=bias_sb[:, o : o + 1],
                              scale=scale,
                          )
                      else:
                          nc.vector.tensor_scalar(
                              outT[:, o, j : j + w],
                              ps[:, :w],
                              scale,
                              bias_sb[:, o : o + 1],
                              mybir.AluOpType.mult,
                              mybir.AluOpType.add,
                          )
                      drain_idx[0] += 1

              def vaug_chunk(c0, pc, idx):
                  ps = ps_proj.tile([128, 512], F32, tag="psp",
                                    name="ps")
                  for ki in range(2):
                      nc.tensor.matmul(
                          ps[:pc, 0 : H * 33],
                          xT_v[:, ki, c0 : c0 + pc],
                          wTaug_bf[:, ki, :],
                          start=(ki == 0),
                          stop=False,
                      )
                  nc.tensor.matmul(
                      ps[:pc, 0 : H * 33],
                      ones_bf[:, :pc],
                      baug_bf,
                      start=False,
                      stop=True,
                  )
                  drain(vpa[idx][:pc, :], ps[:pc, 0 : H * 33])

              q_pairs = [[(c, 128), (c + 1, 128)] for c in range(0, 8, 2)]
              kv_pairs = q_pairs + [[(8, 32)]]
              # work items: (kind, payload); emitted so a transpose pair is
              # always in flight between dependent projection slices.
              work = []
              for p in q_pairs:
                  work.append(("t", (rawq, xT_q, p)))
              for p in kv_pairs:
                  work.append(("t", (rawk, xT_k, p)))
              for p in kv_pairs:
                  work.append(("t", (rawv, xT_v, p)))
              for j in range(0, T, 256):
                  work.append(("pq", j))
              for j in range(0, 1024, 256):
                  work.append(("pk", j))
              work.append(("pk_tail", 1024))
              kv_chunks = [(c * 128, 128) for c in range(8)] + [(1024, 32)]
              for idx, (c0, pc) in enumerate(kv_chunks):
                  work.append(("v", (c0, pc, idx)))

              # schedule: run transposes in order, inserting each projection
              # item as soon as its inputs' transposes have been emitted.
              t_items = [w for w in work if w[0] == "t"]
              # number of t-items that must precede: q slice j needs q pairs
              # up to (j+256)/256; k slice needs 4 q pairs + ...; v chunk all.
              def prereq(item):
                  kind, pl = item
                  if kind == "pq":
                      return (pl + 256) // 256
                  if kind == "pk":
                      return 4 + (pl + 256) // 256
                  if kind == "pk_tail":
                      return 9
                  if kind == "v":
                      c0, pc, idx = pl
                      return 9 + (c0 + pc + 127) // 256 + 1
                  return 0
              p_items = sorted([w for w in work if w[0] != "t"],
                               key=prereq)
              emitted_t = 0
              pi = 0
              for t_item in t_items:
                  transpose_pair(*t_item[1])
                  emitted_t += 1
                  while pi < len(p_items) and prereq(p_items[pi]) + o[
                          "front_lag"] <= emitted_t:
                      kind, pl = p_items[pi]
                      if kind == "pq":
                          proj_slice(xT_q, qpT, pl, 256, b2_sb, 1.0)
                      elif kind == "pk":
                          proj_slice(xT_k, kpT, pl, 256, b2s_sb, SCALING)
                      elif kind == "pk_tail":
                          proj_slice(xT_k, kpT, 1024, 32, b2s_sb, SCALING)
                      else:
                          vaug_chunk(*pl)
                      pi += 1
              while pi < len(p_items):
                  kind, pl = p_items[pi]
                  if kind == "pq":
                      proj_slice(xT_q, qpT, pl, 256, b2_sb, 1.0)
                  elif kind == "pk":
                      proj_slice(xT_k, kpT, pl, 256, b2s_sb, SCALING)
                  elif kind == "pk_tail":
                      proj_slice(xT_k, kpT, 1024, 32, b2s_sb, SCALING)
                  else:
                      vaug_chunk(*pl)
                  pi += 1

              # ---- scores (S^T windowed, bf16) + exp + mask ---------------
              # PSUM layout: [128 (sig,jj), 4 (hr -> bank), 128 (ht,64win)].
              aw = [
                  aw_pool.tile([128, 4, 128], BF16, tag=f"aw{c}",
                               name=f"aw{c}")
                  for c in range(NKTILE)
              ]
              ofin_all = oall_pool.tile([128, NQT, H, 32], F32, tag="oall")

              # ---- out' matmuls + normalize -------------------------------
              def out_tile(t, psf):
                  _ = psf
                  po = ps_o.tile([128, 2, H, 64], F32, tag="pso", name="po")
                  def po_ap(r0, rn, mi, h, wn):
                      return po[r0:r0 + rn, mi, h, 0:wn]
                  if o["m64"]:
                      # kv block m=4t+j covers q rows 32(j-1):32(j+1) of this
                      # tile. Odd j are 64-aligned -> one M=64 matmul; even j
                      # split into M=32 halves (tile col positions must be
                      # 0/64 for 64-wide tiles). j parity -> bank; writes
                      # within a bank are row-disjoint (no concurrent RMW).
                      for h in range(H):
                          hr, ht = h % 4, h // 4
                          # (j, out_row0, rows, lhs_half_col, width)
                          pieces = [
                              (0, 0, 32, 32, 32),
                              (1, 0, 64, 0, 64),
                              (2, 32, 32, 0, 32),
                              (2, 64, 32, 32, 32),
                              (3, 64, 64, 0, 64),
                              (4, 96, 32, 0, 32),
                          ]
                          for j, r0, rn, half, wm in pieces:
                              m = 4 * t + j
                              c, sig = m // 4, m % 4
                              lhsT = aw[c][
                                  32 * sig : 32 * sig + 32, hr,
                                  64 * ht + half : 64 * ht + half + wm,
                              ]
                              rhs = vpa[c][
                                  32 * sig : 32 * sig + 32,
                                  33 * h : 33 * h + 33
                              ]
                              nc.tensor.matmul(
                                  po_ap(r0, rn, j % 2, h, 33),
                                  lhsT,
                                  rhs,
                                  start=True,
                                  stop=True,
                                  tile_position=(32 * sig, r0 if rn == 64
                                                 else r0),
                              )
                  else:
                    for gi in range(4):
                      g = 4 * t + gi
                      for h in range(H):
                          hr, ht = h % 4, h // 4
                          for mi, m in enumerate((g, g + 1)):
                              c, sig = m // 4, m % 4
                              half = 32 if m == g else 0
                              lhsT = aw[c][
                                  32 * sig : 32 * sig + 32, hr,
                                  64 * ht + half : 64 * ht + half + 32,
                              ]
                              rhs = vpa[c][
                                  32 * sig : 32 * sig + 32, 33 * h : 33 * h + 33
                              ]
                              nc.tensor.matmul(
                                  po_ap(32 * gi, 32, mi, h, 33),
                                  lhsT,
                                  rhs,
                                  start=True,
                                  stop=True,
                                  tile_position=(32 * sig, 32 * gi),
                              )
                  def po_all(mi):
                      return po[:, mi, :, 0:33]
                  osum = ofin_pool.tile([128, H, 33], F32, tag="osum")
                  if o["dve_2psum"]:
                      nc.vector.scalar_tensor_tensor(
                          out=osum,
                          in0=po_all(0),
                          scalar=1.0,
                          in1=po_all(1),
                          op0=mybir.AluOpType.mult,
                          op1=mybir.AluOpType.add,
                      )
                  else:
                      pb_sb = ofin_pool.tile([128, H, 33], F32, tag="pb_sb")
                      nc.scalar.activation(
                          pb_sb, po_all(1), mybir.ActivationFunctionType.Copy
                      )
                      nc.vector.scalar_tensor_tensor(
                          out=osum,
                          in0=po_all(0),
                          scalar=1.0,
                          in1=pb_sb,
                          op0=mybir.AluOpType.mult,
                          op1=mybir.AluOpType.add,
                      )
                  rec = ofin_pool.tile([128, H], F32, tag="rec")
                  nc.vector.reciprocal(rec, osum[:, :, 32])
                  rec_b = bass.AP(
                      tensor=rec.tensor,
                      offset=rec.offset,
                      ap=[rec.ap[0], [rec.ap[1][0], H], [0, 32]],
                  )
                  (nc.gpsimd if o["pool_ofin"] else nc.vector).tensor_mul(
                      ofin_all[:, t], osum[:, :, 0:32], rec_b)

              psf_prev = [None]
              if o["pingpong"]:
                  pss_persist = ps_s.tile([128, 4, 512], F32, tag="pss",
                                          name="pss_persist")
              for c in range(NKTILE):
                  nsig = 4 if c < NKTILE - 1 else NB - 4 * c
                  if o["pingpong"]:
                      off = 256 * (c % 2)
                      ps = pss_persist[:, :, off:off + 128]
                      psf = ps
                  else:
                      psf = ps_s.tile([128, 4, 128], F32, tag="pss",
                                      name="psf",
                                      padded_shape=[128, 4, o["ps_s_pad"]])
                      ps = psf
                  if c == NKTILE - 1:
                      nc.vector.memset(ps[:, :, 0:128], 0.0)
                  for sig in range(nsig):
                      m = 4 * c + sig
                      for h in range(H):
                          hr, ht = h % 4, h // 4
                          lhsT = kpT[32 * hr : 32 * hr + 32, ht,
                                     32 * m : 32 * m + 32]
                          if m == 0:
                              rhs = qpT[32 * hr : 32 * hr + 32, ht, 0:32]
                              outap = ps[32 * sig : 32 * sig + 32, hr,
                                         64 * ht + 32 : 64 * ht + 64]
                          elif m == NB - 1:
                              rhs = qpT[
                                  32 * hr : 32 * hr + 32, ht,
                                  32 * (m - 1) : 32 * m
                              ]
                              outap = ps[32 * sig : 32 * sig + 32, hr,
                                         64 * ht : 64 * ht + 32]
                          else:
                              rhs = qpT[
                                  32 * hr : 32 * hr + 32, ht,
                                  32 * (m - 1) : 32 * (m + 1),
                              ]
                              outap = ps[32 * sig : 32 * sig + 32, hr,
                                         64 * ht : 64 * ht + 64]
                          nc.tensor.matmul(
                              outap,
                              lhsT,
                              rhs,
                              start=True,
                              stop=True,
                              tile_position=(32 * hr, 32 * sig),
                          )
                  # zero never-written PSUM regions so exp sees finite values
                  if c == 0:
                      nc.vector.memset(ps[0:32, :, 0:32], 0.0)
                      nc.vector.memset(ps[0:32, :, 64:96], 0.0)
                  ex = aw_pool.tile([128, 4, 128], BF16, tag="ex", bufs=3)
                  mi = 0 if 0 < c < NKTILE - 1 else (1 if c == 0 else 2)
                  mvw = masks_sb[:, mi, :].rearrange("p (r w) -> p r w", r=4)
                  if o["split_exp"]:
                      # halve exp/mask grain so DVE masking overlaps ACT exp
                      for hh in (0, 2):
                          nc.scalar.activation(
                              ex[:, hh:hh + 2, :], ps[:, hh:hh + 2, 0:128],
                              mybir.ActivationFunctionType.Exp)
                          nc.vector.tensor_mul(
                              aw[c][:, hh:hh + 2, :], ex[:, hh:hh + 2, :],
                              mvw[:, hh:hh + 2, :])
                  else:
                      nc.scalar.activation(ex, ps[:, :, 0:128],
                                           mybir.ActivationFunctionType.Exp)
                      mask_eng = (nc.gpsimd if (o["pool_mask"] and c % 2 == 0)
                                  else nc.vector)
                      mask_eng.tensor_mul(aw[c], ex, mvw)
                  if c >= 1:
                      out_tile(c - 1, psf)
                  psf_prev[0] = psf

              # ---- single batched store -----------------------------------
              nc.sync.dma_start(
                  out=out_d.ap().rearrange("(t p) e -> p t e", p=128),
                  in_=ofin_all.rearrange("p t h w -> p t (h w)"),
              )

    nc.compile()
    return nc


_NC_CACHE = None


def _get_nc():
    global _NC_CACHE
    if _NC_CACHE is None:
        _NC_CACHE = build_program()
    return _NC_CACHE


def make_in_maps(query, key, value, W, b):
    query = np.asarray(query, np.float32)
    key = np.asarray(key, np.float32)
    value = np.asarray(value, np.float32)
    W = np.asarray(W, np.float32)
    b = np.asarray(b, np.float32)

    cblob = _make_const_blob(W, b)

    qf = query.reshape(B * N, E)
    kf = key.reshape(B * N, E)
    vf = value.reshape(B * N, E)
    shards_per_b = NCORES // B
    in_maps = []
    for c in range(NCORES):
        s0 = c * T
        halo0 = s0 - 32
        if c % shards_per_b == 0:
            halo_k = np.zeros((32, E), np.float32)
            halo_v = np.zeros((32, E), np.float32)
        else:
            halo_k = kf[halo0:s0]
            halo_v = vf[halo0:s0]
        in_maps.append(
            {
                "q": np.ascontiguousarray(qf[s0 : s0 + T]),
                "k": np.ascontiguousarray(np.concatenate([halo_k, kf[s0 : s0 + T]])),
                "v": np.ascontiguousarray(np.concatenate([halo_v, vf[s0 : s0 + T]])),
                "cblob": cblob,
            }
        )
    return in_maps


def kernel(query, key, value, W, b):
    nc = _get_nc()
    in_maps = make_in_maps(query, key, value, W, b)
    res = run_bass_kernel_spmd(nc, in_maps, list(range(NCORES)))
    out = np.concatenate([res.results[c]["out"] for c in range(NCORES)], axis=0)
    return out.reshape(B, N, E).astype(np.float32)


# revision 34
# speedup vs baseline: 1.6998x; 1.2552x over previous
"""Sliding-window (tau=32) multi-head attention block with shared qkv projection,
distributed over 8 trn2 NeuronCores.

Sharding: data/sequence-parallel over the flattened (batch, token) axis —
8 shards of 1024 tokens. Each core receives its k/v slice with a 32-row
front halo (zeros at batch start), so projecting the concatenated buffer
reproduces the reference's pad-then-project semantics exactly (incl. bias).

Perf structure:
- batched DMAs: one load per raw tensor (+tails), one const blob, one store.
- f32 PE transposes whose PSUM drains cast to bf16 for free; projections,
  scores, exp, mask, and out' matmuls all in bf16 (1 cyc/row at any free
  size, vs f32r's 4 cyc/row below 256 — the small windowed matmuls dominate
  PE; bf16 also avoids f32r's stored-format rounding copies).
"""

import numpy as np
import ml_dtypes

import concourse.bacc as bacc
import concourse.bass as bass
import concourse.tile as tile
from concourse import mybir
from concourse.bass_utils import run_bass_kernel_spmd

B, N, E = 2, 4096, 256
H, TAU = 8, 32
HD = E // H
SCALING = HD**-0.5

NCORES = 8
T = B * N // NCORES  # 1024 q tokens per core
KT = T + 32  # kv rows incl. 32-row front halo
NB = KT // 32  # 33 kv blocks of 32
NQT = T // 128  # 8 q tiles of 128
NKTILE = (NB + 3) // 4  # 9 kpos tiles of (up to) 4 blocks

F32 = mybir.dt.float32
F32R = mybir.dt.float32r
BF16 = mybir.dt.bfloat16

# const blob column offsets (f32 units; bf16 payloads packed 2-per-lane)
_IDENTB0 = 0                              # ident [128,128] bf16 -> 64
_WTB0 = _IDENTB0 + 64                     # wT [2,128,256] bf16 -> 256
_WTAUGB0 = _WTB0 + 256                    # wTaug [2,128,264] bf16 -> 264
_MASK0 = _WTAUGB0 + 264                   # masks [3,128,512] bf16 -> 768
_B20 = _MASK0 + 768
_B2S0 = _B20 + 2
_ONESB0 = _B2S0 + 2                       # ones row [1,128] bf16 -> 64
_BAUGB0 = _ONESB0 + 64                    # baug row [1,264] bf16 -> 132
_IDENTF0 = _BAUGB0 + 132                  # ident [128,128] f32
_CBLOB = _IDENTF0 + 128


def _host_constants():
    """Band masks in S^T window coords: rows jj (kpos within block), cols ii
    (q within the 64-wide window); valid iff ii - jj in [0, 31]."""
    jj = np.arange(32)[:, None]
    ii = np.arange(64)[None, :]
    band = ((ii - jj >= 0) & (ii - jj <= 31)).astype(np.float32)  # [32, 64]
    band128 = np.tile(band, (4, 1))  # [128, 64]
    normal = np.repeat(band128[:, None, :], H, axis=1)  # [128, H, 64]
    first = normal.copy()
    # kpos tile 0, block m=0 (partitions 0:32): left window half (q block -1)
    # does not exist.
    first[0:32, :, 0:32] = 0.0
    last = np.zeros_like(normal)
    # kpos tile 8 holds only block m=32 (partitions 0:32); only its left
    # window half (q block 31) exists.
    last[0:32, :, 0:32] = np.repeat(band[:, 0:32][:, None, :], H, axis=1)
    masks = np.stack([normal.reshape(128, H * 64),
                      first.reshape(128, H * 64),
                      last.reshape(128, H * 64)])  # [3, 128, 512]
    return masks.astype(np.float32)


def _pack_bf16(a):
    """Pack a bf16 array (last axis even) into f32 lanes, little-endian."""
    a = np.ascontiguousarray(a.astype(ml_dtypes.bfloat16))
    u = a.view(np.uint16)
    u = u.reshape(*u.shape[:-1], u.shape[-1] // 2, 2)
    return np.ascontiguousarray(u).view(np.uint32)[..., 0].view(np.float32)


def _make_const_blob(W, b):
    WT = np.ascontiguousarray(W.T).astype(np.float32)  # [e_in, e_out]
    wT = WT.reshape(2, 128, 256)
    # augmented V weights: per-head 33-wide column groups, ones col slot = 0
    WTaug = np.zeros((256, H * 33), np.float32)
    for h in range(H):
        WTaug[:, 33 * h : 33 * h + 32] = WT[:, 32 * h : 32 * h + 32]
    wTaug = WTaug.reshape(2, 128, H * 33)
    b_aug = np.zeros((H * 33,), np.float32)
    for h in range(H):
        b_aug[33 * h : 33 * h + 32] = b[32 * h : 32 * h + 32]
        b_aug[33 * h + 32] = 1.0
    b2 = b.reshape(2, 128).astype(np.float32)

    blob = np.zeros((128, _CBLOB), np.float32)
    blob[:, _IDENTB0:_IDENTB0 + 64] = _pack_bf16(np.eye(128, dtype=np.float32))
    blob[:, _WTB0:_WTB0 + 256] = _pack_bf16(
        wT.transpose(1, 0, 2).reshape(128, 512))
    blob[:, _WTAUGB0:_WTAUGB0 + 264] = _pack_bf16(
        wTaug.transpose(1, 0, 2).reshape(128, 528))
    blob[:, _MASK0:_MASK0 + 768] = _pack_bf16(
        _host_constants().transpose(1, 0, 2).reshape(128, 1536))
    blob[:, _B20:_B20 + 2] = b2.T
    blob[:, _B2S0:_B2S0 + 2] = (SCALING * b2).T
    blob[0, _ONESB0:_ONESB0 + 64] = _pack_bf16(np.ones((1, 128), np.float32))
    blob[0, _BAUGB0:_BAUGB0 + 132] = _pack_bf16(b_aug.reshape(1, -1))
    blob[:, _IDENTF0:_IDENTF0 + 128] = np.eye(128, dtype=np.float32)
    return blob


def build_program(stage=4, reps=1, opts=None):
    _ = stage
    o = {"raw_bufs": 1, "xT_bufs": 1, "pool_mask": False,
         "ps_proj_bufs": 2, "ps_s_bufs": 1, "ps_s_pad": 512,
         "front_lag": 99, "dve_2psum": False, "pool_ofin": False,
         "pingpong": True, "pool_cast": False, "cast_whole": False,
         "m64": False, "xbar": False, "split_exp": False}
    if opts:
        o.update(opts)
    nc = bacc.Bacc("TRN2", target_bir_lowering=False)

    q_d = nc.dram_tensor("q", [T, E], F32, kind="ExternalInput")
    k_d = nc.dram_tensor("k", [KT, E], F32, kind="ExternalInput")
    v_d = nc.dram_tensor("v", [KT, E], F32, kind="ExternalInput")
    cb_d = nc.dram_tensor("cblob", [128, _CBLOB], F32, kind="ExternalInput")
    out_d = nc.dram_tensor("out", [T, E], F32, kind="ExternalOutput")

    with tile.TileContext(nc) as tc:
        with (
            tc.tile_pool(name="consts", bufs=1) as consts,
            tc.tile_pool(name="raw", bufs=o["raw_bufs"]) as raw_pool,
            tc.tile_pool(name="xT", bufs=o["xT_bufs"]) as xT_pool,
            tc.tile_pool(name="proj", bufs=1) as proj_pool,
            tc.tile_pool(name="aw", bufs=1) as aw_pool,
            tc.tile_pool(name="ofin", bufs=4) as ofin_pool,
            tc.tile_pool(name="oall", bufs=1) as oall_pool,
            tc.tile_pool(name="ps_proj", bufs=o["ps_proj_bufs"], space="PSUM") as ps_proj,
            tc.tile_pool(name="ps_s", bufs=o["ps_s_bufs"], space="PSUM") as ps_s,
            tc.tile_pool(name="ps_o", bufs=1, space="PSUM") as ps_o,
        ):
            # ---- constants: one DMA + bitcast views -----------------------
            blob = consts.tile([128, _CBLOB], F32)
            nc.sync.dma_start(out=blob, in_=cb_d.ap())
            ident = blob[:, _IDENTB0:_IDENTB0 + 64].bitcast(BF16)
            ident_f = blob[:, _IDENTF0:_IDENTF0 + 128]
            masks_sb = blob[:, _MASK0:_MASK0 + 768].bitcast(BF16).rearrange(
                "p (i w) -> p i w", i=3)  # [128, 3, 512] bf16
            b2_sb = blob[:, _B20:_B20 + 2]
            b2s_sb = blob[:, _B2S0:_B2S0 + 2]
            wT_bf = blob[:, _WTB0:_WTB0 + 256].bitcast(BF16).rearrange(
                "p (k e) -> p k e", k=2)
            wTaug_bf = blob[:, _WTAUGB0:_WTAUGB0 + 264].bitcast(BF16).rearrange(
                "p (k e) -> p k e", k=2)
            ones_bf = blob[0:1, _ONESB0:_ONESB0 + 64].bitcast(BF16)
            baug_bf = blob[0:1, _BAUGB0:_BAUGB0 + 132].bitcast(BF16)

            for _rep in range(reps):
              _ = _rep  # noqa
              # ---- batched raw loads --------------------------------------
              rawq = raw_pool.tile([128, NQT, E], F32, tag="rawq")
              rawk = raw_pool.tile([128, 9, E], F32, tag="rawk")
              rawv = raw_pool.tile([128, 9, E], F32, tag="rawv")
              nc.sync.dma_start(
                  out=rawq, in_=q_d.ap().rearrange("(c p) e -> p c e", p=128))
              nc.sync.dma_start(
                  out=rawk[:, 0:8, :],
                  in_=k_d.ap()[0:1024].rearrange("(c p) e -> p c e", p=128))
              nc.sync.dma_start(out=rawk[0:32, 8, :], in_=k_d.ap()[1024:KT])
              nc.sync.dma_start(
                  out=rawv[:, 0:8, :],
                  in_=v_d.ap()[0:1024].rearrange("(c p) e -> p c e", p=128))
              nc.sync.dma_start(out=rawv[0:32, 8, :], in_=v_d.ap()[1024:KT])

              # ---- PE transpose -> xT (f32r) ------------------------------
              xT_q = xT_pool.tile([128, 2, T], BF16, tag="xTq")
              xT_k = xT_pool.tile([128, 2, KT], BF16, tag="xTk")
              xT_v = xT_pool.tile([128, 2, KT], BF16, tag="xTv")

              qpT = proj_pool.tile([128, 2, T], BF16, tag="qpT")
              kpT = proj_pool.tile([128, 2, KT], BF16, tag="kpT")
              vpa = [
                  proj_pool.tile([128, H * 33], BF16, tag=f"vpa{i}",
                                 name=f"vpa{i}")
                  for i in range(9)
              ]

              # Front phase: interleave transposes with projections so PE has
              # independent work while ACT/DVE drain PSUM (drain latency would
              # otherwise stall the 2-buffer psp rotation every tile).
              drain_idx = [0]

              def drain(dst, src):
                  if drain_idx[0] % 2 == 0:
                      nc.scalar.activation(
                          dst, src, mybir.ActivationFunctionType.Copy)
                  else:
                      nc.vector.tensor_copy(dst, src)
                  drain_idx[0] += 1

              bfraw_pool_tiles = {}

              def transpose_pair(raw, xT, pair):
                  base = pair[0][0] * 128
                  tot = sum(pc for _, pc in pair)
                  if o["xbar"]:
                      # cast to bf16 on ACT/DVE, then XBAR DMA transpose on
                      # the idle SP queue: out[p, o, j] = in[j, 128o + p].
                      key = id(raw)
                      if key not in bfraw_pool_tiles:
                          bfraw_pool_tiles[key] = raw_pool.tile(
                              [128, 9, E], BF16,
                              tag=f"bfr{len(bfraw_pool_tiles)}", name="bfr")
                      rbf = bfraw_pool_tiles[key]
                      c0 = pair[0][0]
                      nch = len(pair)
                      if drain_idx[0] % 2 == 0:
                          nc.scalar.activation(
                              rbf[:, c0:c0 + nch, :], raw[:, c0:c0 + nch, :],
                              mybir.ActivationFunctionType.Copy)
                      else:
                          nc.vector.tensor_copy(
                              rbf[:, c0:c0 + nch, :], raw[:, c0:c0 + nch, :])
                      drain_idx[0] += 1
                      for c, pc in pair:
                          nc.sync.dma_start_transpose(
                              xT[:, :, 128 * c : 128 * c + pc],
                              rbf[:pc, c, :],
                          )
                      return
                  if not o["pool_cast"]:
                      # f32 transpose; the PSUM drain does the bf16 cast free
                      pt = ps_proj.tile([128, 512], F32, tag="psp",
                                        name="pt").rearrange(
                          "p (a b) -> p a b", a=2)
                      for j, (c, pc) in enumerate(pair):
                          rt = raw[:, c, :]
                          for oo in range(2):
                              nc.tensor.transpose(
                                  pt[:, oo, 128 * j : 128 * j + pc],
                                  rt[:pc, 128 * oo : 128 * oo + 128],
                                  ident_f[:pc, :pc],
                              )
                      drain(xT[:, :, base : base + tot], pt[:, :, :tot])
                      return
                  key = id(raw)
                  if key not in bfraw_pool_tiles:
                      bfraw_pool_tiles[key] = raw_pool.tile(
                          [128, 9, E], BF16, tag=f"bfr{len(bfraw_pool_tiles)}",
                          name="bfr")
                  rbf = bfraw_pool_tiles[key]
                  pt = ps_proj.tile([128, 1024], BF16, tag="psp",
                                    name="pt")[:, 0:512].rearrange(
                      "p (a b) -> p a b", a=2)
                  # cast on the (otherwise idle) GPSIMD: one op per tensor
                  # (coarse, but launches amortized; pipelines across reps)
                  c0 = pair[0][0]
                  nch = len(pair)
                  if o["cast_whole"]:
                      if c0 == 0:
                          nch_all = raw.shape[1]
                          nc.gpsimd.tensor_copy(
                              rbf[:, 0:nch_all, :], raw[:, 0:nch_all, :])
                  else:
                      nc.gpsimd.tensor_copy(
                          rbf[:, c0:c0 + nch, :], raw[:, c0:c0 + nch, :])
                  for j, (c, pc) in enumerate(pair):
                      rt = rbf[:, c, :]
                      for oo in range(2):
                          nc.tensor.transpose(
                              pt[:, oo, 128 * j : 128 * j + pc],
                              rt[:pc, 128 * oo : 128 * oo + 128],
                              ident[:pc, :pc],
                          )
                  drain(xT[:, :, base : base + tot], pt[:, :, :tot])

              def proj_slice(xT, outT, j, w, bias_sb, scale):
                  for o in range(2):
                      ps = ps_proj.tile([128, 512], F32, tag="psp",
                                        name="ps")
                      for ki in range(2):
                          nc.tensor.matmul(
                              ps[:, :w],
                              wT_bf[:, ki, 128 * o : 128 * o + 128],
                              xT[:, ki, j : j + w],
                              start=(ki == 0),
                              stop=(ki == 1),
                          )
                      if drain_idx[0] % 2 == 0:
                          nc.scalar.activation(
                              outT[:, o, j : j + w],
                              ps[:, :w],
                              mybir.ActivationFunctionType.Identity,
                              bias=bias_sb[:, o : o + 1],
                              scale=scale,
                          )
                      else:
                          nc.vector.tensor_scalar(
                              outT[:, o, j : j + w],
                              ps[:, :w],
                              scale,
                              bias_sb[:, o : o + 1],
                              mybir.AluOpType.mult,
                              mybir.AluOpType.add,
                          )
                      drain_idx[0] += 1

              def vaug_chunk(c0, pc, idx):
                  ps = ps_proj.tile([128, 512], F32, tag="psp",
                                    name="ps")
                  for ki in range(2):
                      nc.tensor.matmul(
                          ps[:pc, 0 : H * 33],
                          xT_v[:, ki, c0 : c0 + pc],
                          wTaug_bf[:, ki, :],
                          start=(ki == 0),
                          stop=False,
                      )
                  nc.tensor.matmul(
                      ps[:pc, 0 : H * 33],
                      ones_bf[:, :pc],
                      baug_bf,
                      start=False,
                      stop=True,
                  )
                  drain(vpa[idx][:pc, :], ps[:pc, 0 : H * 33])

              q_pairs = [[(c, 128), (c + 1, 128)] for c in range(0, 8, 2)]
              kv_pairs = q_pairs + [[(8, 32)]]
              # work items: (kind, payload); emitted so a transpose pair is
              # always in flight between dependent projection slices.
              work = []
              for p in q_pairs:
                  work.append(("t", (rawq, xT_q, p)))
              for p in kv_pairs:
                  work.append(("t", (rawk, xT_k, p)))
              for p in kv_pairs:
                  work.append(("t", (rawv, xT_v, p)))
              for j in range(0, T, 256):
                  work.append(("pq", j))
              for j in range(0, 1024, 256):
                  work.append(("pk", j))
              work.append(("pk_tail", 1024))
              kv_chunks = [(c * 128, 128) for c in range(8)] + [(1024, 32)]
              for idx, (c0, pc) in enumerate(kv_chunks):
                  work.append(("v", (c0, pc, idx)))

              # schedule: run transposes in order, inserting each projection
              # item as soon as its inputs' transposes have been emitted.
              t_items = [w for w in work if w[0] == "t"]
              # number of t-items that must precede: q slice j needs q pairs
              # up to (j+256)/256; k slice needs 4 q pairs + ...; v chunk all.
              def prereq(item):
                  kind, pl = item
                  if kind == "pq":
                      return (pl + 256) // 256
                  if kind == "pk":
                      return 4 + (pl + 256) // 256
                  if kind == "pk_tail":
                      return 9
                  if kind == "v":
                      c0, pc, idx = pl
                      return 9 + (c0 + pc + 127) // 256 + 1
                  return 0
              p_items = sorted([w for w in work if w[0] != "t"],
                               key=prereq)
              emitted_t = 0
              pi = 0
              for t_item in t_items:
                  transpose_pair(*t_item[1])
                  emitted_t += 1
                  while pi < len(p_items) and prereq(p_items[pi]) + o[
                          "front_lag"] <= emitted_t:
                      kind, pl = p_items[pi]
                      if kind == "pq":
                          proj_slice(xT_q, qpT, pl, 256, b2_sb, 1.0)
                      elif kind == "pk":
                          proj_slice(xT_k, kpT, pl, 256, b2s_sb, SCALING)
                      elif kind == "pk_tail":
                          proj_slice(xT_k, kpT, 1024, 32, b2s_sb, SCALING)
                      else:
                          vaug_chunk(*pl)
                      pi += 1
              while pi < len(p_items):
                  kind, pl = p_items[pi]
                  if kind == "pq":
                      proj_slice(xT_q, qpT, pl, 256, b2_sb, 1.0)
                  elif kind == "pk":
                      proj_slice(xT_k, kpT, pl, 256, b2s_sb, SCALING)
                  elif kind == "pk_tail":
                      proj_slice(xT_k, kpT, 1024, 32, b2s_sb, SCALING)
                  else:
                      vaug_chunk(*pl)
                  pi += 1

              # ---- scores (S^T windowed, bf16) + exp + mask ---------------
              # PSUM layout: [128 (sig,jj), 4 (hr -> bank), 128 (ht,64win)].
              aw = [
                  aw_pool.tile([128, 4, 128], BF16, tag=f"aw{c}",
                               name=f"aw{c}")
                  for c in range(NKTILE)
              ]
              ofin_all = oall_pool.tile([128, NQT, H, 32], F32, tag="oall")

              # ---- out' matmuls + normalize -------------------------------
              def out_tile(t, psf):
                  _ = psf
                  po = ps_o.tile([128, 2, H, 64], F32, tag="pso", name="po")
                  def po_ap(r0, rn, mi, h, wn):
                      return po[r0:r0 + rn, mi, h, 0:wn]
                  if o["m64"]:
                      # kv block m=4t+j covers q rows 32(j-1):32(j+1) of this
                      # tile. Odd j are 64-aligned -> one M=64 matmul; even j
                      # split into M=32 halves (tile col positions must be
                      # 0/64 for 64-wide tiles). j parity -> bank; writes
                      # within a bank are row-disjoint (no concurrent RMW).
                      for h in range(H):
                          hr, ht = h % 4, h // 4
                          # (j, out_row0, rows, lhs_half_col, width)
                          pieces = [
                              (0, 0, 32, 32, 32),
                              (1, 0, 64, 0, 64),
                              (2, 32, 32, 0, 32),
                              (2, 64, 32, 32, 32),
                              (3, 64, 64, 0, 64),
                              (4, 96, 32, 0, 32),
                          ]
                          for j, r0, rn, half, wm in pieces:
                              m = 4 * t + j
                              c, sig = m // 4, m % 4
                              lhsT = aw[c][
                                  32 * sig : 32 * sig + 32, hr,
                                  64 * ht + half : 64 * ht + half + wm,
                              ]
                              rhs = vpa[c][
                                  32 * sig : 32 * sig + 32,
                                  33 * h : 33 * h + 33
                              ]
                              nc.tensor.matmul(
                                  po_ap(r0, rn, j % 2, h, 33),
                                  lhsT,
                                  rhs,
                                  start=True,
                                  stop=True,
                                  tile_position=(32 * sig, r0 if rn == 64
                                                 else r0),
                              )
                  else:
                    for gi in range(4):
                      g = 4 * t + gi
                      for h in range(H):
                          hr, ht = h % 4, h // 4
                          for mi, m in enumerate((g, g + 1)):
                              c, sig = m // 4, m % 4
                              half = 32 if m == g else 0
                              lhsT = aw[c][
                                  32 * sig : 32 * sig + 32, hr,
                                  64 * ht + half : 64 * ht + half + 32,
                              ]
                              rhs = vpa[c][
                                  32 * sig : 32 * sig + 32, 33 * h : 33 * h + 33
                              ]
                              nc.tensor.matmul(
                                  po_ap(32 * gi, 32, mi, h, 33),
                                  lhsT,
                                  rhs,
                                  start=True,
                                  stop=True,
                                  tile_position=(32 * sig, 32 * gi),
                              )
                  def po_all(mi):
                      return po[:, mi, :, 0:33]
                  osum = ofin_pool.tile([128, H, 33], F32, tag="osum")
                  if o["dve_2psum"]:
                      nc.vector.scalar_tensor_tensor(
                          out=osum,
                          in0=po_all(0),
                          scalar=1.0,
                          in1=po_all(1),
                          op0=mybir.AluOpType.mult,
                          op1=mybir.AluOpType.add,
                      )
                  else:
                      pb_sb = ofin_pool.tile([128, H, 33], F32, tag="pb_sb")
                      nc.scalar.activation(
                          pb_sb, po_all(1), mybir.ActivationFunctionType.Copy
                      )
                      nc.vector.scalar_tensor_tensor(
                          out=osum,
                          in0=po_all(0),
                          scalar=1.0,
                          in1=pb_sb,
                          op0=mybir.AluOpType.mult,
                          op1=mybir.AluOpType.add,
                      )
                  rec = ofin_pool.tile([128, H], F32, tag="rec")
                  nc.vector.reciprocal(rec, osum[:, :, 32])
                  rec_b = bass.AP(
                      tensor=rec.tensor,
                      offset=rec.offset,
                      ap=[rec.ap[0], [rec.ap[1][0], H], [0, 32]],
                  )
                  (nc.gpsimd if o["pool_ofin"] else nc.vector).tensor_mul(
                      ofin_all[:, t], osum[:, :, 0:32], rec_b)

              psf_prev = [None]
              if o["pingpong"]:
                  pss_persist = ps_s.tile([128, 4, 512], F32, tag="pss",
                                          name="pss_persist")
              for c in range(NKTILE):
                  nsig = 4 if c < NKTILE - 1 else NB - 4 * c
                  if o["pingpong"]:
                      off = 256 * (c % 2)
                      ps = pss_persist[:, :, off:off + 128]
                      psf = ps
                  else:
                      psf = ps_s.tile([128, 4, 128], F32, tag="pss",
                                      name="psf",
                                      padded_shape=[128, 4, o["ps_s_pad"]])
                      ps = psf
                  if c == NKTILE - 1:
                      nc.vector.memset(ps[:, :, 0:128], 0.0)
                  for sig in range(nsig):
                      m = 4 * c + sig
                      for h in range(H):
                          hr, ht = h % 4, h // 4
                          lhsT = kpT[32 * hr : 32 * hr + 32, ht,
                                     32 * m : 32 * m + 32]
                          if m == 0:
                              rhs = qpT[32 * hr : 32 * hr + 32, ht, 0:32]
                              outap = ps[32 * sig : 32 * sig + 32, hr,
                                         64 * ht + 32 : 64 * ht + 64]
                          elif m == NB - 1:
                              rhs = qpT[
                                  32 * hr : 32 * hr + 32, ht,
                                  32 * (m - 1) : 32 * m
                              ]
                              outap = ps[32 * sig : 32 * sig + 32, hr,
                                         64 * ht : 64 * ht + 32]
                          else:
                              rhs = qpT[
                                  32 * hr : 32 * hr + 32, ht,
                                  32 * (m - 1) : 32 * (m + 1),
                              ]
                              outap = ps[32 * sig : 32 * sig + 32, hr,
                                         64 * ht : 64 * ht + 64]
                          nc.tensor.matmul(
                              outap,
                              lhsT,
                              rhs,
                              start=True,
                              stop=True,
                              tile_position=(32 * hr, 32 * sig),
                          )
                  # zero never-written PSUM regions so exp sees finite values
                  if c == 0:
                      nc.vector.memset(ps[0:32, :, 0:32], 0.0)
                      nc.vector.memset(ps[0:32, :, 64:96], 0.0)
                  ex = aw_pool.tile([128, 4, 128], BF16, tag="ex", bufs=3)
                  mi = 0 if 0 < c < NKTILE - 1 else (1 if c == 0 else 2)
                  mvw = masks_sb[:, mi, :].rearrange("p (r w) -> p r w", r=4)
                  if o["split_exp"]:
                      # halve exp/mask grain so DVE masking overlaps ACT exp
                      for hh in (0, 2):
                          nc.scalar.activation(
                              ex[:, hh:hh + 2, :], ps[:, hh:hh + 2, 0:128],
                              mybir.ActivationFunctionType.Exp)
                          nc.vector.tensor_mul(
                              aw[c][:, hh:hh + 2, :], ex[:, hh:hh + 2, :],
                              mvw[:, hh:hh + 2, :])
                  else:
                      nc.scalar.activation(ex, ps[:, :, 0:128],
                                           mybir.ActivationFunctionType.Exp)
                      mask_eng = (nc.gpsimd if (o["pool_mask"] and c % 2 == 0)
                                  else nc.vector)
                      mask_eng.tensor_mul(aw[c], ex, mvw)
                  if c >= 1:
                      out_tile(c - 1, psf)
                  psf_prev[0] = psf

              # ---- single batched store -----------------------------------
              nc.sync.dma_start(
                  out=out_d.ap().rearrange("(t p) e -> p t e", p=128),
                  in_=ofin_all.rearrange("p t h w -> p t (h w)"),
              )

    nc.compile()
    return nc


_NC_CACHE = None


def _get_nc():
    global _NC_CACHE
    if _NC_CACHE is None:
        _NC_CACHE = build_program()
    return _NC_CACHE


def make_in_maps(query, key, value, W, b):
    query = np.asarray(query, np.float32)
    key = np.asarray(key, np.float32)
    value = np.asarray(value, np.float32)
    W = np.asarray(W, np.float32)
    b = np.asarray(b, np.float32)

    cblob = _make_const_blob(W, b)

    qf = query.reshape(B * N, E)
    kf = key.reshape(B * N, E)
    vf = value.reshape(B * N, E)
    shards_per_b = NCORES // B
    in_maps = []
    for c in range(NCORES):
        s0 = c * T
        halo0 = s0 - 32
        if c % shards_per_b == 0:
            halo_k = np.zeros((32, E), np.float32)
            halo_v = np.zeros((32, E), np.float32)
        else:
            halo_k = kf[halo0:s0]
            halo_v = vf[halo0:s0]
        in_maps.append(
            {
                "q": np.ascontiguousarray(qf[s0 : s0 + T]),
                "k": np.ascontiguousarray(np.concatenate([halo_k, kf[s0 : s0 + T]])),
                "v": np.ascontiguousarray(np.concatenate([halo_v, vf[s0 : s0 + T]])),
                "cblob": cblob,
            }
        )
    return in_maps


def kernel(query, key, value, W, b):
    nc = _get_nc()
    in_maps = make_in_maps(query, key, value, W, b)
    res = run_bass_kernel_spmd(nc, in_maps, list(range(NCORES)))
    out = np.concatenate([res.results[c]["out"] for c in range(NCORES)], axis=0)
    return out.reshape(B, N, E).astype(np.float32)
